# revision 8
# baseline (speedup 1.0000x reference)
"""Trainium2 Bass kernel for CrossAttentionFusion.

Reference computation (B=4, C=256, Cs=256, CI=128, H=W=64, N=M=4096):
    q = Wq @ x + bq; k = Wk @ z + bk; v = Wv @ z + bv
    att = softmax(q^T k, axis=m);  out = gamma * (v @ att^T) + x

Sharding: 8 cores = 4 batches x 2 query-halves (2048 queries each).
Each core holds full K/V for its batch; weights replicated.

Per-core design (v2):
- Energies are computed *transposed* (eT[m, n], m on partitions) so both the
  QK^T and AV matmuls contract along the partition dim - no transposes of
  the attention matrix anywhere.
- The energy matmul runs in fp8 DoubleRow mode (2x PE rate): contraction
  slots (p, i) hold q duplicated across i and k split hi/lo across i, so
  e = q8.(k_hi + k_lo) - exact in k, fp8-rounded in q only (~0.3 nats of
  energy noise; rel err ~5e-3, validated vs fp64 in numpy).
- The V projection also runs DoubleRow fp8 (z and Wv as fp8, 2x128
  contraction per pass); v then stored bf16 for the bf16 AV matmul.
  The AV matmul stays bf16: fp8 p would overflow e4m3 (energies reach ~86
  and rows have genuine 5-7 sigma spikes; no cheap per-row max exists).
- Softmax runs without max subtraction (bf16 p; exp overflows only past 88).
- The softmax denominator is accumulated by the DMA engines (dma_start with
  accum_op=add, SBUF->SBUF) into 3 accumulators plus one DVE-owned
  accumulator - this removes the old DVE/GPSIMD add chain almost entirely.
- gamma is folded into the reciprocal broadcast (gones row = gamma*ones),
  so the epilogue is tensor_tensor(out_ps * rg) + stt(+gamma*bv + x).
- The PE stream is software-pipelined: energy matmuls for group g+1 are
  emitted before the AV matmuls of group g; projections for later tiles
  are woven into tile 0's group loop.
"""
import sys

if "/opt/trn_rl_repo" not in sys.path:
    sys.path.insert(0, "/opt/trn_rl_repo")

import ml_dtypes
import numpy as np

B, C, CS, CI, H, W = 4, 256, 256, 128, 64, 64
N = H * W            # 4096 keys/values per batch
NQ = N // 2          # 2048 queries per core
N_CORES = 8
NT = NQ // 512       # 4 query tiles of 512
MC = N // 128        # 32 m-chunks of 128
NG = MC // 2         # 16 exp groups of 2 m-chunks

BF16 = ml_dtypes.bfloat16
E4M3 = ml_dtypes.float8_e4m3
_CACHE = {}


def _build(with_shift=False):
    from concourse import bacc, mybir
    from concourse.tile import TileContext

    f32 = mybir.dt.float32
    f32r = mybir.dt.float32r
    bf16 = mybir.dt.bfloat16
    fp8 = mybir.dt.float8e4
    EXP = mybir.ActivationFunctionType.Exp
    ADD = mybir.AluOpType.add
    SUB = mybir.AluOpType.subtract
    MULT = mybir.AluOpType.mult
    DR = mybir.MatmulPerfMode.DoubleRow

    nc = bacc.Bacc("TRN2", num_devices=N_CORES, debug=False)

    xm = nc.dram_tensor("xm", [C, NQ], f32r, kind="ExternalInput")
    xmb = nc.dram_tensor("xmb", [C, NQ], bf16, kind="ExternalInput")
    zf = nc.dram_tensor("zf", [CS, N], bf16, kind="ExternalInput")
    zf8 = nc.dram_tensor("zf8", [128, 2 * N], fp8, kind="ExternalInput")
    wqt = nc.dram_tensor("wqt", [C, CI], bf16, kind="ExternalInput")
    wkt = nc.dram_tensor("wkt", [C, CI], bf16, kind="ExternalInput")
    wvt8 = nc.dram_tensor("wvt8", [128, 2 * C], fp8, kind="ExternalInput")
    bq = nc.dram_tensor("bq", [CI, 1], f32, kind="ExternalInput")
    bk = nc.dram_tensor("bk", [CI, 1], f32, kind="ExternalInput")
    gbv = nc.dram_tensor("gbv", [C, 1], f32, kind="ExternalInput")
    eshift = (nc.dram_tensor("eshift", [128, 1], f32, kind="ExternalInput")
              if with_shift else None)
    onesd = nc.dram_tensor("onesd", [128, 1], bf16, kind="ExternalInput")
    gones = nc.dram_tensor("gones", [1, 128], bf16, kind="ExternalInput")
    out = nc.dram_tensor("out", [C, NQ], f32, kind="ExternalOutput")

    with TileContext(nc) as tc:
        with tc.tile_pool(name="const", bufs=1) as cpool, \
             tc.tile_pool(name="big", bufs=1) as bpool, \
             tc.tile_pool(name="vtp", bufs=NG) as vpool, \
             tc.tile_pool(name="work", bufs=3) as wpool, \
             tc.tile_pool(name="ps", bufs=2, space="PSUM") as ps:

            # ---- big activations first on the sync DMA queue.
            # DMAs are chained into priority bands (later bands wait on
            # earlier ones) so the first-needed pieces get full HBM
            # bandwidth instead of sharing it with the bulk.
            from concourse.bass import _add_dep_helper

            zf_t = [bpool.tile([128, N], bf16, tag=f"zf{i}", name=f"zf{i}")
                    for i in range(2)]
            xm_t = [bpool.tile([128, NQ], bf16, tag=f"xm{i}", name=f"xm{i}")
                    for i in range(2)]
            prev_band = []
            for h in range(4):
                hs = slice(h * 1024, (h + 1) * 1024)
                band = []
                for i in range(2):
                    dma = nc.sync.dma_start(zf_t[i][:, hs],
                                            zf.ap()[i * 128:(i + 1) * 128, hs])
                    for p in prev_band:
                        _add_dep_helper(dma.ins, p.ins, sync=True,
                                        reason="dma priority band")
                    band.append(dma)
                prev_band = band
            xm_first = []
            for i in range(2):
                dma = nc.scalar.dma_start(xm_t[i][:, 0:512],
                                          xmb.ap()[i * 128:(i + 1) * 128, 0:512])
                xm_first.append(dma)
            for i in range(2):
                dma = nc.scalar.dma_start(xm_t[i][:, 512:NQ],
                                          xmb.ap()[i * 128:(i + 1) * 128, 512:NQ])
                for p in xm_first:
                    _add_dep_helper(dma.ins, p.ins, sync=True,
                                    reason="dma priority band")

            # ---- weights / consts / fp8 z on the gpsimd DMA queue --------
            wkt_t = [cpool.tile([128, CI], bf16, tag=f"wkt{i}", name=f"wkt{i}")
                     for i in range(2)]
            wqt_t = [cpool.tile([128, CI], bf16, tag=f"wqt{i}", name=f"wqt{i}")
                     for i in range(2)]
            wvt8_t = cpool.tile([128, 2, C], fp8, tag="wvt8")
            zf8_t = bpool.tile([128, 2, N], fp8, tag="zf8")
            bq_t = cpool.tile([CI, 1], f32, tag="bq")
            bk_t = cpool.tile([CI, 1], f32, tag="bk")
            gbv_t = [cpool.tile([128, 1], f32, tag=f"gbv{i}", name=f"gbv{i}")
                     for i in range(2)]
            ones_t = cpool.tile([128, 1], bf16, tag="ones")
            gones_t = cpool.tile([1, 128], bf16, tag="gones")
            eshift_t = (cpool.tile([128, 1], f32, tag="eshift", name="eshift_t")
                        if with_shift else None)
            for i in range(2):
                nc.gpsimd.dma_start(wkt_t[i][:], wkt.ap()[i * 128:(i + 1) * 128, :])
            nc.gpsimd.dma_start(bk_t[:], bk.ap())
            nc.gpsimd.dma_start(wvt8_t[:], wvt8.ap())
            # zf8 halves: first-needed m range first
            for i in range(2):
                nc.gpsimd.dma_start(zf8_t[:, i, 0:2048],
                                    zf8.ap()[:, i * N:i * N + 2048])
            for i in range(2):
                nc.gpsimd.dma_start(wqt_t[i][:], wqt.ap()[i * 128:(i + 1) * 128, :])
            nc.gpsimd.dma_start(bq_t[:], bq.ap())
            nc.gpsimd.dma_start(ones_t[:], onesd.ap())
            nc.gpsimd.dma_start(gones_t[:], gones.ap())
            if with_shift:
                nc.gpsimd.dma_start(eshift_t[:], eshift.ap())
            for i in range(2):
                nc.gpsimd.dma_start(gbv_t[i][:], gbv.ap()[i * 128:(i + 1) * 128, :])
            for i in range(2):
                nc.gpsimd.dma_start(zf8_t[:, i, 2048:N],
                                    zf8.ap()[:, i * N + 2048:(i + 1) * N])

            k2_t = bpool.tile([128, 2, N], fp8, tag="k2")
            q2_t = bpool.tile([128, 2, NQ], fp8, tag="q2")
            vt_t = [vpool.tile([128, 512], bf16, tag="vt", name=f"vt{_}")
                    for _ in range(NG)]

            # Projections, interleaved K/Q/VT so PE never waits on the DVE
            # PSUM->SBUF copies.

            def emit_k(mt):
                # k tile mt covers m [mt*512, (mt+1)*512); writes k2 hi+lo
                pk = ps.tile([128, 1024], f32, tag="e", name=f"pk{mt}")
                sl = slice(mt * 512, (mt + 1) * 512)
                nc.tensor.matmul(pk[:, 0:512], wkt_t[0][:], zf_t[0][:, sl],
                                 start=True, stop=False)
                nc.tensor.matmul(pk[:, 0:512], wkt_t[1][:], zf_t[1][:, sl],
                                 start=False, stop=True)
                nc.vector.tensor_scalar_add(k2_t[:, 0, sl], pk[:, 0:512], bk_t[:])
                nc.vector.scalar_tensor_tensor(
                    k2_t[:, 1, sl], pk[:, 0:512], bk_t[:], k2_t[:, 0, sl],
                    op0=ADD, op1=SUB)

            def emit_q(qt):
                pq = ps.tile([128, 1024], f32, tag="e", name=f"pq{qt}")
                sl = slice(qt * 512, (qt + 1) * 512)
                nc.tensor.matmul(pq[:, 0:512], wqt_t[0][:], xm_t[0][:, sl],
                                 start=True, stop=False)
                nc.tensor.matmul(pq[:, 0:512], wqt_t[1][:], xm_t[1][:, sl],
                                 start=False, stop=True)
                nc.vector.tensor_scalar_add(q2_t[:, 0, sl], pq[:, 0:512], bq_t[:])
                # duplicate q8 into the second DoubleRow slot via DMA
                nc.sync.dma_start(q2_t[:, 1, sl], q2_t[:, 0, sl])

            def emit_vt(g2):
                # VT pair tile g2 = chunks (2*g2, 2*g2+1); DoubleRow fp8 proj
                pv = ps.tile([128, 1024], f32, tag="e", name=f"pv{g2}")
                for j in range(2):
                    mc = 2 * g2 + j
                    sl = slice(mc * 128, (mc + 1) * 128)
                    nc.tensor.matmul(pv[:, j * 256:(j + 1) * 256],
                                     zf8_t[:, :, sl], wvt8_t[:],
                                     start=True, stop=True, perf_mode=DR)
                nc.vector.tensor_copy(vt_t[g2][:], pv[:, 0:512])

            # Prologue: just enough projections for the first groups of
            # attention; the rest are woven into nt0's group loop below.
            emit_k(0)
            emit_k(1)
            emit_q(0)
            emit_vt(0)

            # ---- attention -----------------------------------------------
            def emit_et(g, nsl):
                e_ps = ps.tile([128, 1024], f32, tag="e", name=f"e{g}")
                for j in range(2):
                    mc = 2 * g + j
                    nc.tensor.matmul(
                        e_ps[:, j * 512:(j + 1) * 512],
                        k2_t[:, :, mc * 128:(mc + 1) * 128],
                        q2_t[:, :, nsl], start=True, stop=True, perf_mode=DR)
                return e_ps

            def make_tail(nt, out_ps, accw, acc3, x_sbs):
                # Deferred softmax tail + epilogue for query-tile nt.
                def tail():
                    nsl = slice(nt * 512, (nt + 1) * 512)
                    nc.vector.tensor_add(accw[0][:], accw[0][:], accw[1][:])
                    nc.vector.tensor_add(accw[0][:], accw[0][:], accw[2][:])
                    nc.vector.tensor_add(acc3[:], acc3[:], accw[0][:, 0:512])
                    nc.vector.tensor_add(acc3[:], acc3[:], accw[0][:, 512:1024])
                    sums_ps = ps.tile([1, 512], f32, tag="e", name="sums_ps")
                    nc.tensor.matmul(sums_ps[:], ones_t[:], acc3[:],
                                     start=True, stop=True)
                    recip = wpool.tile([1, 512], f32, tag="recip")
                    nc.vector.reciprocal_approx_fast(recip[:], sums_ps[:])
                    recip_b = wpool.tile([1, 512], bf16, tag="recip_b")
                    nc.vector.tensor_copy(recip_b[:], recip[:])
                    rg_ps = ps.tile([128, 512], f32, tag="e", name="rg_ps")
                    nc.tensor.matmul(rg_ps[:], gones_t[:], recip_b[:],
                                     start=True, stop=True)
                    rg_sb = wpool.tile([128, 512], f32, tag="rg")
                    nc.vector.tensor_copy(rg_sb[:], rg_ps[:])
                    # epilogue: out = out_unnorm * gamma/sums + gamma*bv + x
                    for oc in range(2):
                        csl = slice(oc * 128, (oc + 1) * 128)
                        t_sb = wpool.tile([128, 512], f32, tag="t")
                        nc.vector.tensor_tensor(
                            t_sb[:], out_ps[oc][:], rg_sb[:], op=MULT)
                        f_sb = wpool.tile([128, 512], f32, tag="f")
                        nc.vector.scalar_tensor_tensor(
                            f_sb[:], t_sb[:], gbv_t[oc][:], x_sbs[oc][:],
                            op0=ADD, op1=ADD)
                        nc.sync.dma_start(out.ap()[csl, nsl], f_sb[:])
                return tail

            for nt in range(NT):
                nsl = slice(nt * 512, (nt + 1) * 512)
                out_ps = [ps.tile([128, 512], f32, tag=f"o{oc}",
                                  name=f"ops{oc}", bufs=2) for oc in range(2)]
                x_sbs = []
                for oc in range(2):
                    csl = slice(oc * 128, (oc + 1) * 128)
                    x_sb = wpool.tile([128, 512], f32r, tag="x", bufs=4,
                                      name=f"x{oc}")
                    nc.sync.dma_start(x_sb[:], xm.ap()[csl, nsl])
                    x_sbs.append(x_sb)
                # softmax-denominator accumulators: 3 double-wide DMA-fed
                # chains + 1 DVE-fed
                accw = [wpool.tile([128, 1024], bf16, tag=f"accw{i}", bufs=2,
                                   name=f"accw{i}_{nt}") for i in range(3)]
                acc3 = wpool.tile([128, 512], bf16, tag="acc3", bufs=2,
                                  name=f"acc3_{nt}")

                if nt == 0:
                    e_next = emit_et(0, nsl)
                for g in range(NG):
                    e_cur = e_next
                    p_t = wpool.tile([128, 1024], bf16, tag="p", bufs=14)
                    nc.scalar.activation(
                        p_t[:], e_cur[:], EXP,
                        bias=eshift_t[:] if with_shift else 0.0)
                    if g + 1 < NG:
                        e_next = emit_et(g + 1, nsl)
                    elif nt + 1 < NT:
                        # cross-boundary lookahead: next tile's first energy
                        # matmuls run while this tile's tail drains
                        e_next = emit_et(0, slice((nt + 1) * 512,
                                                  (nt + 2) * 512))
                    if nt == 0:
                        if 0 <= g < 6:
                            emit_k(g + 2)
                        if g <= 14:
                            emit_vt(g + 1)
                    if g == 8 and nt < NT - 1:
                        emit_q(nt + 1)
                    for j in range(2):
                        mc = 2 * g + j
                        pair, par = mc // 2, mc % 2
                        for oc in range(2):
                            nc.tensor.matmul(
                                out_ps[oc][:],
                                vt_t[pair][:, par * 256 + oc * 128:
                                           par * 256 + (oc + 1) * 128],
                                p_t[:, j * 512:(j + 1) * 512],
                                start=(mc == 0), stop=(mc == MC - 1))
                    # softmax denominator: groups g%4==3 are accumulated by
                    # DVE (both halves); other groups go whole ([128,1024])
                    # to one of 3 double-wide accumulators via gpsimd
                    # accum-DMAs. First touch of each acc initializes.
                    a = g % 4
                    if a == 3:
                        for j in range(2):
                            psl = p_t[:, j * 512:(j + 1) * 512]
                            if g < 4:
                                nc.vector.tensor_copy(acc3[:], psl) if j == 0 \
                                    else nc.vector.tensor_add(acc3[:], acc3[:], psl)
                            else:
                                nc.vector.tensor_add(acc3[:], acc3[:], psl)
                    else:
                        if g < 3:
                            nc.gpsimd.dma_start(accw[a][:], p_t[:])
                        else:
                            nc.gpsimd.dma_start(accw[a][:], p_t[:],
                                                accum_op=ADD)
                make_tail(nt, out_ps, accw, acc3, x_sbs)()

    nc.compile()
    return nc


def _get_nc(with_shift=False):
    key = ("nc", with_shift)
    if key not in _CACHE:
        _CACHE[key] = _build(with_shift)
    return _CACHE[key]


def kernel(x_main, z_p, Wq, bq, Wk, bk, Wv, bv, gamma, _trace=False):
    from concourse import bass_utils

    xm_full = np.ascontiguousarray(np.asarray(x_main, np.float32)).reshape(B, C, N)
    zf_full_f32 = np.asarray(z_p, np.float32).reshape(B, CS, N)
    zf_full = zf_full_f32.astype(BF16)
    g = float(np.float32(np.asarray(gamma).reshape(-1)[0]))

    # Softmax runs without per-row max subtraction; that is safe while
    # max(energy) stays below ~86 (fp32/bf16 exp overflow near 88.7).
    # Check a subsample first; only if borderline, compute the exact global
    # max (BLAS, ~0.5s) and fall back to a globally-shifted exp variant.
    Wq32 = np.asarray(Wq, np.float32)
    Wk32 = np.asarray(Wk, np.float32)
    bq32 = np.asarray(bq, np.float32).reshape(1, CI, 1)
    bk32 = np.asarray(bk, np.float32).reshape(1, CI, 1)
    qs = np.einsum("oc,bcn->bon", Wq32, xm_full[:, :, ::16]) + bq32
    ks_ = np.einsum("oc,bcm->bom", Wk32, zf_full_f32[:, :, ::16]) + bk32
    sample_max = float(np.einsum("bon,bom->bnm", qs, ks_).max())
    shift = 0.0
    if sample_max * 1.6 > 85.0:
        q_all = np.einsum("oc,bcn->bon", Wq32, xm_full) + bq32
        k_all = np.einsum("oc,bcm->bom", Wk32, zf_full_f32) + bk32
        true_max = max(
            float((q_all[b].T @ k_all[b]).max()) for b in range(B))
        if true_max > 84.0:
            shift = true_max - 80.0
    nc = _get_nc(with_shift=shift != 0.0)

    # fp8 z in DoubleRow layout: zf8[p, i*N+m] = z[i*128+p, m]
    zf8_full = np.ascontiguousarray(
        zf_full_f32.reshape(B, 2, 128, N).transpose(0, 2, 1, 3)
        .reshape(B, 128, 2 * N).astype(E4M3))
    # Wv in DoubleRow layout: wvt8[p, i*C+c] = Wv[c, i*128+p]
    Wv32 = np.asarray(Wv, np.float32)
    wvt8 = np.ascontiguousarray(
        Wv32.T.reshape(2, 128, C).transpose(1, 0, 2).reshape(128, 2 * C)
        .astype(E4M3))

    common = {
        "wqt": np.ascontiguousarray(np.asarray(Wq, np.float32).T.astype(BF16)),
        "wkt": np.ascontiguousarray(np.asarray(Wk, np.float32).T.astype(BF16)),
        "wvt8": wvt8,
        "bq": np.asarray(bq, np.float32).reshape(CI, 1),
        "bk": np.asarray(bk, np.float32).reshape(CI, 1),
        "gbv": (np.float32(g) * np.asarray(bv, np.float32)).reshape(C, 1),
        "onesd": np.ones((128, 1), BF16),
        "gones": np.full((1, 128), g, BF16),
    }
    if shift != 0.0:
        common["eshift"] = np.full((128, 1), -shift, np.float32)
    in_maps = []
    for core in range(N_CORES):
        b, half = divmod(core, 2)
        in_maps.append({
            "xm": np.ascontiguousarray(xm_full[b][:, half * NQ:(half + 1) * NQ]),
            "xmb": np.ascontiguousarray(
                xm_full[b][:, half * NQ:(half + 1) * NQ].astype(BF16)),
            "zf": np.ascontiguousarray(zf_full[b]),
            "zf8": zf8_full[b],
            **common,
        })

    res = bass_utils.run_bass_kernel_spmd(
        nc, in_maps, core_ids=list(range(N_CORES)), trace=_trace)

    out = np.empty((B, C, N), np.float32)
    for core in range(N_CORES):
        b, half = divmod(core, 2)
        out[b][:, half * NQ:(half + 1) * NQ] = res.results[core]["out"]
    if _trace:
        _CACHE["last_result"] = res
    return out.reshape(B, C, H, W)
